# revision 3
# baseline (speedup 1.0000x reference)
"""Trainium2 Bass kernel for variational CF embedding model (nn_CF_50362786513123).

Data-parallel over 8 NeuronCores: tables replicated, batch dim of x sharded.
Host fuses bias_table [V,2] and entity_table [V,40] into one [V,42] table so a
single per-partition indirect DMA (the HW granularity: 128 random rows per
instruction) fetches both. The work is chunked so ACT/DVE compute for chunk c
overlaps the gather stream of later chunks on GpSimd.
"""
import sys
import types

sys.path.insert(0, "/opt/trn_rl_repo")

import numpy as np

# ---- problem constants (hardcoded per spec) ----
D = 20
VOCAB = 1_100_000
B = 65536
NCORES = 8
P = 128                            # SBUF partitions
FLAT = 2 * B                       # 131072 rows total
FPC = FLAT // NCORES               # 16384 flat rows per core
K = FPC // P                       # 128 index columns per partition
M = K // 2                         # 64 pairs per partition
BPC = B // NCORES                  # 8192 pairs per core
R = 2 * D + 2                      # fused row length: [bias(2), entity(40)]
NCH = 8                            # pipeline chunks
KC = K // NCH                      # 16 index columns per chunk
MC = KC // 2                       # 8 pairs per chunk per partition

TRACE = False                      # test harness can set kernel.TRACE = True
LAST_EXEC_NS = None
LAST_RESULT = None

_prog = None


def _install_ntff_hook():
    """Install the axon NTFF profile hook bass_utils expects (absent in image)."""
    try:
        import antenv.axon_hooks  # noqa: F401
        return
    except ImportError:
        pass
    try:
        from trn_agent_boot.trn_boot import _ntff_profile_via_ctypes
        hook = _ntff_profile_via_ctypes("/opt/axon/libaxon_pjrt.so")
    except Exception:
        hook = None
    m = types.ModuleType("antenv.axon_hooks")
    m._hook = hook
    m.set_axon_ntff_profile_hook = lambda h: setattr(m, "_hook", h)
    m.get_axon_ntff_profile_hook = lambda: m._hook
    sys.modules["antenv.axon_hooks"] = m


def _build():
    import concourse.bass as bass
    import concourse.bacc as bacc
    import concourse.tile as tile
    import concourse.mybir as mybir

    f32 = mybir.dt.float32
    AF = mybir.ActivationFunctionType
    SQ5 = float(np.sqrt(0.5))      # fold the 0.5 of the KL into Square's scale
    SQE = float(np.exp(0.5))       # fold the -0.5 of the KL into Ln's scale

    nc = bacc.Bacc("TRN2", target_bir_lowering=False, num_devices=NCORES)

    ftab_t = nc.dram_tensor("ftab", [VOCAB, R], f32, kind="ExternalInput")
    idx_t = nc.dram_tensor("idx", [P, K], mybir.dt.int32, kind="ExternalInput")
    epse_t = nc.dram_tensor("eps_e", [P, K * D], f32, kind="ExternalInput")
    epsb_t = nc.dram_tensor("eps_b", [P, K], f32, kind="ExternalInput")
    gb_t = nc.dram_tensor("gb", [1], f32, kind="ExternalInput")
    al_t = nc.dram_tensor("alpha", [1], f32, kind="ExternalInput")

    pred_t = nc.dram_tensor("pred", [P, M], f32, kind="ExternalOutput")
    klb_t = nc.dram_tensor("kl_b", [P, K], f32, kind="ExternalOutput")
    kle_t = nc.dram_tensor("kl_e", [P, K], f32, kind="ExternalOutput")
    std_t = nc.dram_tensor("std", [1], f32, kind="ExternalOutput")

    with tile.TileContext(nc) as tc:
        with tc.tile_pool(name="main", bufs=1) as pool:
            it = pool.tile([P, K], mybir.dt.int32)
            nc.sync.dma_start(it[:], idx_t[:])
            Ee = pool.tile([P, K * D], f32)
            nc.sync.dma_start(Ee[:], epse_t[:])
            Eb = pool.tile([P, K], f32)
            nc.sync.dma_start(Eb[:], epsb_t[:])
            gbt = pool.tile([P, 1], f32)
            nc.sync.dma_start(gbt[:], gb_t[:].to_broadcast((P, 1)))
            alt = pool.tile([1, 1], f32)
            nc.sync.dma_start(alt[:], al_t[:, None])

            for c in range(NCH):
                k0 = c * KC

                # ---- gather: 128 random fused rows per instruction
                F = pool.tile([P, KC * R], f32, tag=f"F{c}")
                for j in range(KC):
                    k = k0 + j
                    nc.gpsimd.indirect_dma_start(
                        out=F[:, j * R:(j + 1) * R], out_offset=None,
                        in_=ftab_t[:],
                        in_offset=bass.IndirectOffsetOnAxis(ap=it[:, k:k + 1], axis=0))

                F3 = F[:].rearrange("p (k e) -> p k e", e=R)
                mu_b = F3[:, :, 0:1]                 # [P, KC, 1] stride R
                sraw_b = F3[:, :, 1:2]
                mu_e = F3[:, :, 2:2 + D]             # [P, KC, D] stride R
                sraw_e = F3[:, :, 2 + D:2 + 2 * D]

                Eec = Ee[:, k0 * D:(k0 + KC) * D]
                Ebc = Eb[:, k0:k0 + KC]

                # softplus(sraw) = Ln(Exp(sraw) + 1); table rows are ~0.1*N(0,1)
                # so Exp cannot overflow here
                EX = pool.tile([P, KC * D], f32, tag=f"EX{c}")
                nc.scalar.activation(EX[:].rearrange("p (k e) -> p k e", e=D), sraw_e, AF.Exp)
                Ss = pool.tile([P, KC * D], f32, tag=f"Ss{c}")
                nc.scalar.activation(Ss[:], EX[:], AF.Ln, bias=1.0)

                # entity samples: ent = mu + s * eps
                T = pool.tile([P, KC * D], f32, tag=f"T{c}")
                nc.vector.tensor_mul(T[:], Ss[:], Eec)
                ENT = pool.tile([P, KC * D], f32, tag=f"ENT{c}")
                nc.vector.tensor_add(
                    ENT[:].rearrange("p (k e) -> p k e", e=D),
                    T[:].rearrange("p (k e) -> p k e", e=D), mu_e)

                # pair product + dot over D
                ENT4 = ENT[:].rearrange("p (m u e) -> p m u e", u=2, e=D)
                PR = pool.tile([P, MC * D], f32, tag=f"PR{c}")
                nc.vector.tensor_mul(
                    PR[:].rearrange("p (m e) -> p m e", e=D), ENT4[:, :, 0, :], ENT4[:, :, 1, :])
                DOT = pool.tile([P, MC], f32, tag=f"DOT{c}")
                nc.vector.reduce_sum(
                    out=DOT[:], in_=PR[:].rearrange("p (m e) -> p m e", e=D),
                    axis=mybir.AxisListType.X)

                # ---- bias path
                EXb = pool.tile([P, KC], f32, tag=f"EXb{c}")
                nc.scalar.activation(EXb[:].rearrange("p (k o) -> p k o", o=1), sraw_b, AF.Exp)
                Sb = pool.tile([P, KC], f32, tag=f"Sb{c}")
                nc.scalar.activation(Sb[:], EXb[:], AF.Ln, bias=1.0)
                Tb = pool.tile([P, KC], f32, tag=f"Tb{c}")
                nc.vector.tensor_mul(Tb[:], Sb[:], Ebc)
                BS = pool.tile([P, KC], f32, tag=f"BS{c}")
                nc.vector.tensor_add(
                    BS[:].rearrange("p (k o) -> p k o", o=1),
                    Tb[:].rearrange("p (k o) -> p k o", o=1), mu_b)
                BSUM = pool.tile([P, MC], f32, tag=f"BSUM{c}")
                nc.vector.reduce_sum(
                    out=BSUM[:], in_=BS[:].rearrange("p (m u) -> p m u", u=2),
                    axis=mybir.AxisListType.X)

                # pred = global_bias + bias pair sum + dot
                PD = pool.tile([P, MC], f32, tag=f"PD{c}")
                nc.vector.tensor_add(PD[:], DOT[:], BSUM[:])
                PDG = pool.tile([P, MC], f32, tag=f"PDG{c}")
                nc.scalar.activation(PDG[:], PD[:], AF.Identity, bias=gbt[:, 0:1])
                nc.sync.dma_start(pred_t[:, c * MC:(c + 1) * MC], PDG[:])

                # ---- kl_bias = 0.5*(s^2 + mu^2) - (ln s + 0.5)
                SQs = pool.tile([P, KC], f32, tag=f"SQs{c}")
                nc.scalar.activation(SQs[:], Sb[:], AF.Square, scale=SQ5)
                SQm = pool.tile([P, KC], f32, tag=f"SQm{c}")
                nc.scalar.activation(SQm[:].rearrange("p (k o) -> p k o", o=1), mu_b, AF.Square, scale=SQ5)
                Ub = pool.tile([P, KC], f32, tag=f"Ub{c}")
                nc.vector.tensor_add(Ub[:], SQs[:], SQm[:])
                LN2b = pool.tile([P, KC], f32, tag=f"LN2b{c}")
                nc.scalar.activation(LN2b[:], Sb[:], AF.Ln, scale=SQE)
                KLB = pool.tile([P, KC], f32, tag=f"KLB{c}")
                nc.vector.tensor_sub(KLB[:], Ub[:], LN2b[:])
                nc.sync.dma_start(klb_t[:, k0:k0 + KC], KLB[:])

                # ---- kl_entity: same per-element then sum over D
                SQse = pool.tile([P, KC * D], f32, tag=f"SQse{c}")
                nc.scalar.activation(SQse[:], Ss[:], AF.Square, scale=SQ5)
                SQme = pool.tile([P, KC * D], f32, tag=f"SQme{c}")
                nc.scalar.activation(SQme[:].rearrange("p (k e) -> p k e", e=D), mu_e, AF.Square, scale=SQ5)
                Ue = pool.tile([P, KC * D], f32, tag=f"Ue{c}")
                nc.vector.tensor_add(Ue[:], SQse[:], SQme[:])
                LN2e = pool.tile([P, KC * D], f32, tag=f"LN2e{c}")
                nc.scalar.activation(LN2e[:], Ss[:], AF.Ln, scale=SQE)
                KROW = pool.tile([P, KC * D], f32, tag=f"KROW{c}")
                nc.vector.tensor_sub(KROW[:], Ue[:], LN2e[:])
                KLE = pool.tile([P, KC], f32, tag=f"KLE{c}")
                nc.vector.reduce_sum(
                    out=KLE[:], in_=KROW[:].rearrange("p (k e) -> p k e", e=D),
                    axis=mybir.AxisListType.X)
                nc.sync.dma_start(kle_t[:, k0:k0 + KC], KLE[:])

            # ---- std_dev = sqrt(1/softplus(alpha)), alpha can be huge (1e9):
            # stable softplus(a) = relu(a) + Ln(Exp(-|a|) + 1)
            ab = pool.tile([1, 1], f32)
            nc.scalar.activation(ab[:], alt[:], AF.Abs)
            en = pool.tile([1, 1], f32)
            nc.scalar.activation(en[:], ab[:], AF.Exp, scale=-1.0)
            l1 = pool.tile([1, 1], f32)
            nc.scalar.activation(l1[:], en[:], AF.Ln, bias=1.0)
            rl = pool.tile([1, 1], f32)
            nc.scalar.activation(rl[:], alt[:], AF.Relu)
            sp = pool.tile([1, 1], f32)
            nc.vector.tensor_add(sp[:], rl[:], l1[:])
            l2 = pool.tile([1, 1], f32)
            nc.scalar.activation(l2[:], sp[:], AF.Ln)
            st = pool.tile([1, 1], f32)
            nc.scalar.activation(st[:], l2[:], AF.Exp, scale=-0.5)
            nc.sync.dma_start(std_t[:, None], st[:])

    nc.compile()
    return nc


def kernel(x, bias_table, entity_table, alpha, global_bias, eps_bias, eps_entity):
    global _prog, LAST_EXEC_NS, LAST_RESULT
    _install_ntff_hook()
    from concourse.bass_utils import run_bass_kernel_spmd

    x = np.asarray(x)
    bias_table = np.asarray(bias_table, dtype=np.float32)
    entity_table = np.asarray(entity_table, dtype=np.float32)
    alpha = np.asarray(alpha, dtype=np.float32).reshape(1)
    global_bias = np.asarray(global_bias, dtype=np.float32).reshape(1)
    eps_bias = np.asarray(eps_bias, dtype=np.float32)
    eps_entity = np.asarray(eps_entity, dtype=np.float32)

    # host-side input prep: fuse the two tables so one gather fetches both rows
    ftab = np.ascontiguousarray(
        np.concatenate([bias_table, entity_table], axis=1))   # [V, 42]

    flat = np.ascontiguousarray(x.reshape(-1).astype(np.int32))          # [2B]
    epsb = np.ascontiguousarray(eps_bias.reshape(FLAT))                  # [2B]
    epse = np.ascontiguousarray(eps_entity.reshape(FLAT, D))             # [2B, D]

    if _prog is None:
        _prog = _build()
    nc = _prog

    in_maps = []
    for c in range(NCORES):
        lo, hi = c * FPC, (c + 1) * FPC
        in_maps.append({
            "ftab": ftab,
            "idx": flat[lo:hi].reshape(P, K),
            "eps_e": epse[lo:hi].reshape(P, K * D),
            "eps_b": epsb[lo:hi].reshape(P, K),
            "gb": global_bias,
            "alpha": alpha,
        })

    res = run_bass_kernel_spmd(nc, in_maps, core_ids=list(range(NCORES)), trace=TRACE)
    LAST_RESULT = res
    LAST_EXEC_NS = res.exec_time_ns

    pred = np.concatenate([res.results[c]["pred"].reshape(BPC) for c in range(NCORES)])
    kl_b = np.concatenate([res.results[c]["kl_b"].reshape(FPC) for c in range(NCORES)])
    kl_e = np.concatenate([res.results[c]["kl_e"].reshape(FPC) for c in range(NCORES)])
    std = res.results[0]["std"].reshape(1)
    return pred, std, kl_b, kl_e
